# revision 26
# baseline (speedup 1.0000x reference)
"""Distributed multi-head causal attention for 8 TRN2 NeuronCores.

Problem: B=4, T=2048, D=2048, H=16 heads of dk=dv=128.
  out = softmax(mask((q@Wq)(k@Wk)^T / sqrt(dk))) @ (v@Wv) @ Wo

Sharding (2D; all per-core asymmetry lives in host-supplied data so the
SPMD graph is identical on every core):
  core c -> batch b = c//2, head-group g = c%2 (heads 8g..8g+7).
  - QKV projections + attention for (batch b, its 8 heads): fully local.
  - Pair AllGather (replica groups [2b, 2b+1]) exchanges the per-head
    attention outputs (merged^T, bf16) per q-chunk.
  - Output projection: each core computes out^T for its batch for HALF
    the output columns (even core: cols 0..1023, odd: 1024..2047).
  Host reassembles: out[b] = concat(outT_2b, outT_2b+1, axis=0).T

Performance structure (v2):
  - All intermediates (Q^T/K^T per head, V natural) stay RESIDENT IN
    SBUF - no DRAM round trip, no attention-phase input DMAs.
  - Two HWDGE rings: weights/cc/output on nc.sync (SP), activation
    streams on nc.scalar (ACT) - no head-of-line blocking.
  - Attention chunks emitted in order (0, 3, 2, 1); each chunk's output
    projection is emitted 2+ chunk-slots after its attention, so every
    pair-AllGather (~25-30us latency) lands long before its consumer.
    Order: att0 att3 att2 wo3 att1 wo2 wo0 wo1.
  - PE never parks: long warmup covers the initial weight DMA, then the
    V/K/Q projections and attention+wo run as one dense matmul stream
    (keeps the HAM clock gate at K=8/8).

Compute is bf16 on TensorE with f32 PSUM accumulation. Softmax skips the
max-subtraction (scores are ~N(0,1); exp is safe in f32) and obtains the
denominators with an extra ones-matmul so everything stays on TensorE;
causal masking multiplies exp(scores) by a 0/1 triangular tile on the
single diagonal-crossing 128x128 sub-block, and the moving free dim of
diagonal-region matmuls is trimmed to the unmasked columns.
"""
import os
import sys
from contextlib import ExitStack

import numpy as np
import ml_dtypes

import concourse.bass as bass
import concourse.mybir as mybir
import concourse.tile as tile
from concourse import bacc
from concourse.bass_utils import run_bass_kernel_spmd

BF16 = mybir.dt.bfloat16
F32 = mybir.dt.float32

B, T, D = 4, 2048, 2048
H, DK, DV = 16, 128, 128
HG = 8                      # heads per core
N_CORES = 8
QC = 512                    # q-chunk (matmul moving free dim)
NQC = T // QC               # 4
NKB = T // 128              # 16 k-blocks
NDC = D // 128              # 16 contraction chunks
SCALE = 1.0 / np.sqrt(DK)
N_WARM = 350                # dummy matmuls to warm the PE clock gate

_KERNEL_CACHE = {}

# Attention chunk emission order: each wo(x) needs its pair-AllGather
# landed; gathers fire right after att(x) and take ~7-35us, so wo(x) is
# scheduled >=2 chunk-slots later (see the plan at the end of
# build_kernel). att3 goes first: its 16-k-block pipeline is deepest,
# hiding the exp/DVE latency while the phase warms up.
ATT_ORDER = (3, 0, 2, 1)


def build_kernel(causal: bool):
    nc = bacc.Bacc("TRN2", num_devices=N_CORES)

    qT = nc.declare_dram_parameter("qT", [D, T], BF16, isOutput=False)
    kT = nc.declare_dram_parameter("kT", [D, T], BF16, isOutput=False)
    vT = nc.declare_dram_parameter("vT", [D, T], BF16, isOutput=False)
    wq = nc.declare_dram_parameter("wq", [D, HG * DK], BF16, isOutput=False)
    wk = nc.declare_dram_parameter("wk", [D, HG * DK], BF16, isOutput=False)
    wv = nc.declare_dram_parameter("wv", [D, HG * DV], BF16, isOutput=False)
    wo = nc.declare_dram_parameter("wo", [H * DV, D // 2], BF16, isOutput=False)
    tri = nc.declare_dram_parameter("tri", [128, 128], BF16, isOutput=False)
    if not causal:
        maskT = nc.declare_dram_parameter("maskT", [T, T], BF16, isOutput=False)
    outT = nc.declare_dram_parameter("outT", [D // 2, T], F32, isOutput=True)

    # Collective staging (collectives require DRAM in/out). One pair
    # AllGather per q-chunk: in [128, HG, QC], out [2, 128, HG, QC]
    # (slot 0 = even core's heads = global heads 0..7).
    cc_in = {qc: nc.dram_tensor(f"cc_in_{qc}", [128, HG, QC], BF16)
             for qc in range(NQC)}
    cc_out = {qc: nc.dram_tensor(f"cc_out_{qc}", [2, 128, HG, QC], BF16)
              for qc in range(NQC)}
    cc_warm_in = nc.dram_tensor("cc_warm_in", [128, 8], BF16)
    cc_warm_out = nc.dram_tensor("cc_warm_out", [2, 128, 8], BF16)
    pair_groups = [[0, 1], [2, 3], [4, 5], [6, 7]]

    def kb_start(qc, kb):
        """First unmasked q column (within the chunk) for this k-block."""
        if not causal:
            return 0
        return min(max((kb - 4 * qc) * 128, 0), QC)

    with tile.TileContext(nc) as tc, ExitStack() as top:
        ent = top.enter_context
        consts = ent(tc.tile_pool(name="consts", bufs=1))
        # SBUF-resident per-head projections (live for the whole kernel):
        #   q_all/k_all [128(dk), HG, T] = Q^T/K^T per head
        #   v_all [128(krow), HG, NKB, DV] = V natural per head per k-block
        res_pool = ent(tc.tile_pool(name="res", bufs=1))
        # Phase-1-only pools: the weight ring (two zones, wv->wq reuse)
        # and the double-buffered activation streams. Closed before the
        # attention pools open so their SBUF is recycled.
        proj_stack = ExitStack()
        w_pool = proj_stack.enter_context(tc.tile_pool(name="w", bufs=2))
        x_pool = proj_stack.enter_context(tc.tile_pool(name="xs", bufs=2))

        ones_sb = consts.tile([128, 128], BF16)
        nc.vector.memset(ones_sb, 1.0)
        tri_sb = consts.tile([128, 128], BF16)
        nc.sync.dma_start(out=tri_sb, in_=tri[:])

        # Warm the CC stream: the first collective of a NEFF pays the
        # stream barrier + cold-start (~10-30us extra). Fire a tiny
        # dependency-free pair-AllGather now so all of that happens under
        # the projection phase, not under the first real exchange.
        nc.gpsimd.collective_compute(
            "AllGather",
            mybir.AluOpType.bypass,
            ins=[cc_warm_in[:]],
            outs=[cc_warm_out[:]],
            replica_groups=pair_groups,
        )

        q_all = res_pool.tile([128, HG, T], BF16, name="q_all")
        k_all = res_pool.tile([128, HG, T], BF16, name="k_all")
        v_all = res_pool.tile([128, HG, NKB, DV], BF16, name="v_all")

        def weight_tile(pool):
            return pool.tile([128, NDC, HG * 128], BF16, tag="w", name="wtile")

        def weight_slice(w_sb, w_ext, dci):
            nc.sync.dma_start(
                out=w_sb[:, dci, :], in_=w_ext[dci * 128 : (dci + 1) * 128, :]
            )

        # wv first (feeds the first matmuls), wk lands during V proj.
        wv_sb = weight_tile(w_pool)
        for dci in range(NDC):
            weight_slice(wv_sb, wv, dci)
        wk_sb = weight_tile(w_pool)
        wq_sb = None  # allocated after V proj (reuses wv zone)

        def x_stream(src, qc):
            """[128, NDC, QC] slice of an x^T input, contraction on
            partitions, via the ACT HWDGE ring."""
            xs = x_pool.tile([128, NDC, QC], BF16, tag="xs")
            nc.scalar.dma_start(
                out=xs,
                in_=src[:, qc * QC : (qc + 1) * QC].rearrange(
                    "(o p) f -> p o f", p=128
                ),
            )
            return xs

        # ------------- Phase 1: V, K, Q projections -------------
        # One shared PSUM pool for all three (no pool-boundary bubbles).
        with (
            tc.tile_pool(name="warmps", bufs=1, space="PSUM") as warmps,
            tc.tile_pool(name="ppsum", bufs=4, space="PSUM") as ppsum,
        ):
            # Warm the PE HAM clock gate while the first input DMAs land:
            # dependency-free matmuls on the ones tile into a scratch bank.
            wps = warmps.tile([128, 128], F32)
            for i in range(N_WARM):
                nc.tensor.matmul(
                    wps, lhsT=ones_sb, rhs=ones_sb,
                    start=(i == 0), stop=(i == N_WARM - 1),
                )

            # V natural ([krows, dv], krows on partitions): stationary is
            # the x^T block, the weight columns stream.
            for qv in range(NQC):
                xv = x_stream(vT, qv)
                for dci in (range(4) if qv == 0 else []):
                    weight_slice(wk_sb, wk, dci)
                for kbs in range(4):
                    kb = 4 * qv + kbs
                    if kbs == 3 and qv < NQC - 1:
                        for dci in range(4 * (qv + 1), 4 * (qv + 2)):
                            weight_slice(wk_sb, wk, dci)
                    for nn in range(2):
                        ps = ppsum.tile([128, 512], F32, tag="pp")
                        for dci in range(NDC):
                            nc.tensor.matmul(
                                ps,
                                lhsT=xv[:, dci, kbs * 128 : (kbs + 1) * 128],
                                rhs=wv_sb[:, dci, nn * 512 : (nn + 1) * 512],
                                start=(dci == 0),
                                stop=(dci == NDC - 1),
                            )
                        # drain straight into the resident V tile
                        nc.vector.tensor_copy(
                            out=v_all[:, nn * 4 : (nn + 1) * 4, kb, :], in_=ps
                        )

            # K^T per head ([dk, q]): weight slice stationary, x^T streams.
            wq_sb = weight_tile(w_pool)  # reuses the wv zone
            for qc in range(NQC):
                xs = x_stream(kT, qc)
                for dci in range(4 * qc, 4 * qc + 4):
                    weight_slice(wq_sb, wq, dci)
                for h in range(HG):
                    ps = ppsum.tile([128, QC], F32, tag="pp")
                    for dci in range(NDC):
                        nc.tensor.matmul(
                            ps,
                            lhsT=wk_sb[:, dci, h * 128 : (h + 1) * 128],
                            rhs=xs[:, dci, :],
                            start=(dci == 0),
                            stop=(dci == NDC - 1),
                        )
                    nc.vector.tensor_copy(
                        out=k_all[:, h, qc * QC : (qc + 1) * QC], in_=ps
                    )

            # Q projection, chunks in attention order.
            for qc in ATT_ORDER:
                xs = x_stream(qT, qc)
                for h in range(HG):
                    ps = ppsum.tile([128, QC], F32, tag="pp")
                    for dci in range(NDC):
                        nc.tensor.matmul(
                            ps,
                            lhsT=wq_sb[:, dci, h * 128 : (h + 1) * 128],
                            rhs=xs[:, dci, :],
                            start=(dci == 0),
                            stop=(dci == NDC - 1),
                        )
                    nc.vector.tensor_copy(
                        out=q_all[:, h, qc * QC : (qc + 1) * QC], in_=ps
                    )

        proj_stack.close()

        # ---------- Phase 2+3: attention, pair-AG, output proj ----------
        # wo lands in the SBUF recycled from the weight ring; its 4MB DMA
        # runs under the first attention chunks (first use is emit_wo(3),
        # ~120us into the attention phase).
        wo_pool = ent(tc.tile_pool(name="wop", bufs=1))
        wo_sb = wo_pool.tile([128, NDC, D // 2], BF16, name="wo_sb")
        for dci in range(NDC):
            weight_slice(wo_sb, wo, dci)
        pt_pool = ent(tc.tile_pool(name="pt", bufs=10 if causal else 6))
        racc_pool = ent(tc.tile_pool(name="racc", bufs=4 if causal else 3))
        mstage = ent(tc.tile_pool(name="mstage", bufs=4 if causal else 3))
        rinv_pool = ent(tc.tile_pool(name="rinv", bufs=3 if causal else 2))
        mfq_pool = ent(tc.tile_pool(name="mfq", bufs=2 if causal else 1))
        ob_pool = ent(tc.tile_pool(name="ob", bufs=4))
        gm_pool = ent(tc.tile_pool(name="gm", bufs=2)) if not causal else None
        spsum = ent(tc.tile_pool(name="spsum", bufs=2, space="PSUM"))
        opsum = ent(tc.tile_pool(name="opsum", bufs=2, space="PSUM"))
        rpsum = ent(tc.tile_pool(name="rpsum", bufs=2, space="PSUM"))
        wpsum = ent(tc.tile_pool(name="wpsum", bufs=2, space="PSUM"))

        def load_gm(qc):
            if causal:
                return None
            gm = gm_pool.tile([128, NKB, QC], BF16, tag="gm")
            nc.scalar.dma_start(
                out=gm,
                in_=maskT[:, qc * QC : (qc + 1) * QC].rearrange(
                    "(o p) f -> p o f", p=128
                ),
            )
            return gm

        def att_head(qc, h, gm):
            nkb = 4 * (qc + 1) if causal else NKB
            ngrp = (nkb + 3) // 4
            # Process the diagonal-crossing k-group FIRST: its thin
            # (128..512-wide) s->exp->o chains then overlap the dense
            # full-width blocks instead of bunching at the head's end.
            # PSUM accumulation is order-independent; the group-first
            # block always has j0=0 either way.
            grp_order = ([ngrp - 1] + list(range(ngrp - 1))) if causal else \
                list(range(ngrp))
            kb_order = [4 * g + j for g in grp_order for j in range(4)
                        if 4 * g + j < nkb]
            if True:
                o_ps = opsum.tile([128, QC], F32, tag="opsum")
                r_ps = rpsum.tile([128, QC], F32, tag="rpsum")
                racc = None
                for kbi, kb in enumerate(kb_order):
                    j0 = kb_start(qc, kb)  # first live q col in chunk
                    s_ps = spsum.tile([128, QC], F32, tag="spsum")
                    nc.tensor.matmul(
                        s_ps[:, j0:],
                        lhsT=k_all[:, h, kb * 128 : (kb + 1) * 128],
                        rhs=q_all[:, h, qc * QC + j0 : (qc + 1) * QC],
                        start=True,
                        stop=True,
                    )
                    pt = pt_pool.tile([128, QC], BF16, tag="pt")
                    nc.scalar.activation(
                        out=pt[:, j0:],
                        in_=s_ps[:, j0:],
                        func=mybir.ActivationFunctionType.Exp,
                        scale=float(SCALE),
                    )
                    if causal:
                        if j0 < QC and kb - 4 * qc >= 0:
                            # mask the diagonal-crossing 128 columns
                            nc.vector.tensor_mul(
                                out=pt[:, j0 : j0 + 128],
                                in0=pt[:, j0 : j0 + 128],
                                in1=tri_sb,
                            )
                    else:
                        nc.vector.tensor_mul(out=pt, in0=pt, in1=gm[:, kb, :])
                    nc.tensor.matmul(
                        o_ps[:, j0:],
                        lhsT=v_all[:, h, kb, :],
                        rhs=pt[:, j0:],
                        start=(kbi == 0),
                        stop=(kbi == nkb - 1),
                    )
                    # Denominators: sum groups of 4 exp-blocks on DVE
                    # (bf16), then ONE full-width ones-matmul per group -
                    # a third of the attention PE time used to go to a
                    # per-block ones-matmul. The group's first block always
                    # has j0=0, so racc is fully initialized.
                    if kbi % 4 == 0:
                        racc = racc_pool.tile([128, QC], BF16, tag="racc")
                        nc.vector.tensor_copy(out=racc, in_=pt)
                    else:
                        nc.vector.tensor_add(
                            out=racc[:, j0:], in0=racc[:, j0:], in1=pt[:, j0:]
                        )
                    if kbi % 4 == 3 or kbi == nkb - 1:
                        nc.tensor.matmul(
                            r_ps,
                            lhsT=ones_sb,
                            rhs=racc,
                            start=(kbi // 4 == 0),
                            stop=(kbi // 4 == ngrp - 1),
                        )
                # 1/r on DVE. The exact reciprocal (~6 cycles/elem, 3.4us
                # per head) held the rpsum bank hostage and stalled the PE;
                # the single-instruction NR-seeded approximation (~51 ULP)
                # is ~5x faster. (Scalar-engine Ln+Exp(-x) is no better:
                # exp and ln land in different ACT tables and each head
                # pays two 1.3us ACT_TABLE_LOADs.)
                rinv = rinv_pool.tile([128, QC], F32, tag="rinv")
                nc.vector.reciprocal_approx_fast(out=rinv, in_=r_ps)
                msb = mstage.tile([128, QC], BF16, tag="mstage")
                nc.vector.tensor_mul(out=msb, in0=o_ps, in1=rinv)
                nc.sync.dma_start(out=cc_in[qc][:, h, :], in_=msb)
                if h == HG - 1:
                    nc.gpsimd.collective_compute(
                        "AllGather",
                        mybir.AluOpType.bypass,
                        ins=[cc_in[qc][:]],
                        outs=[cc_out[qc][:]],
                        replica_groups=pair_groups,
                    )

        def wo_load(qc):
            # Prefetch the gathered heads for wo(qc), a full slot early.
            mfq = mfq_pool.tile([128, H, QC], BF16, tag="mfq")
            nc.sync.dma_start(out=mfq[:, :HG, :], in_=cc_out[qc][0])
            nc.sync.dma_start(out=mfq[:, HG:, :], in_=cc_out[qc][1])
            return mfq

        def wo_col(qc, col, mfq):
            w_ps = wpsum.tile([128, QC], F32, tag="wpsum")
            for hv in range(H):
                nc.tensor.matmul(
                    w_ps,
                    lhsT=wo_sb[:, hv, col * 128 : (col + 1) * 128],
                    rhs=mfq[:, hv, :],
                    start=(hv == 0),
                    stop=(hv == H - 1),
                )
            # drain on the Scalar engine - DVE is loaded with the softmax
            # element-wise work, ACT is idle during wo stretches
            ob = ob_pool.tile([128, QC], F32, tag="ob")
            nc.scalar.activation(
                out=ob, in_=w_ps, func=mybir.ActivationFunctionType.Copy
            )
            # outT goes out on the ACT HWDGE ring: the SP ring carries
            # cc_in staging + mfq loads and outT bursts were head-of-line
            # blocking them.
            nc.scalar.dma_start(
                out=outT[
                    col * 128 : (col + 1) * 128,
                    qc * QC : (qc + 1) * QC,
                ],
                in_=ob,
            )

        # Plan: att3 att0 att2 run back-to-back (their gathers fire and
        # land under later chunks); att1's heads interleave with wo3's
        # columns so the exp pipeline never drains ahead of a pure-matmul
        # stretch; remaining wo chunks run ordered by gather-land time.
        NCOL = D // 2 // 128
        gm3 = load_gm(3)
        for h in range(HG):
            att_head(3, h, gm3)
        gm0 = load_gm(0)
        for h in range(HG):
            att_head(0, h, gm0)
        mfq3 = wo_load(3)
        gm2 = load_gm(2)
        for h in range(HG):
            att_head(2, h, gm2)
        mfq0 = wo_load(0)
        gm1 = load_gm(1)
        for h in range(HG):
            att_head(1, h, gm1)
            wo_col(3, h, mfq3)
        mfq2 = wo_load(2)
        for col in range(NCOL):
            wo_col(0, col, mfq0)
        mfq1 = wo_load(1)
        for col in range(NCOL):
            wo_col(2, col, mfq2)
        for col in range(NCOL):
            wo_col(1, col, mfq1)

    nc.compile()
    return nc


def kernel(q, k, v, mask, Wq, Wk, Wv, Wo):
    q = np.asarray(q)
    k = np.asarray(k)
    v = np.asarray(v)
    mask = np.asarray(mask)
    causal = bool(np.array_equal(mask, np.tril(np.ones((T, T), dtype=bool))))

    if causal not in _KERNEL_CACHE:
        _KERNEL_CACHE[causal] = build_kernel(causal)
    nc = _KERNEL_CACHE[causal]

    bf = ml_dtypes.bfloat16
    Wq_b = np.asarray(Wq).astype(bf)
    Wk_b = np.asarray(Wk).astype(bf)
    Wv_b = np.asarray(Wv).astype(bf)
    Wo_b = np.asarray(Wo).astype(bf)
    i = np.arange(128)
    tri_np = (i[None, :] >= i[:, None]).astype(bf)  # tri[k, j] = j >= k
    maskT_np = None if causal else np.ascontiguousarray(mask.T).astype(bf)

    in_maps = []
    for c in range(N_CORES):
        b, g = c // 2, c % 2
        m = {
            "qT": np.ascontiguousarray(q[b].T).astype(bf),
            "kT": np.ascontiguousarray(k[b].T).astype(bf),
            "vT": np.ascontiguousarray(v[b].T).astype(bf),
            "wq": np.ascontiguousarray(Wq_b[:, g * 1024 : (g + 1) * 1024]),
            "wk": np.ascontiguousarray(Wk_b[:, g * 1024 : (g + 1) * 1024]),
            "wv": np.ascontiguousarray(Wv_b[:, g * 1024 : (g + 1) * 1024]),
            "wo": np.ascontiguousarray(Wo_b[:, g * 1024 : (g + 1) * 1024]),
            "tri": tri_np,
        }
        if not causal:
            m["maskT"] = maskT_np
        in_maps.append(m)

    trace = bool(os.environ.get("BASS_KERNEL_TRACE")) and (
        "antenv.axon_hooks" in sys.modules
    )
    res = run_bass_kernel_spmd(nc, in_maps, list(range(N_CORES)), trace=trace)
    if trace and res.exec_time_ns is not None:
        print(f"HW exec time: {res.exec_time_ns} ns")
        kernel.last_exec_time_ns = res.exec_time_ns
        kernel.last_results = res

    out = np.empty((B, T, D), dtype=np.float32)
    for b in range(B):
        top = res.results[2 * b]["outT"]        # cols 0..1023, [1024, 2048]
        bot = res.results[2 * b + 1]["outT"]    # cols 1024..2047
        out[b] = np.concatenate([top, bot], axis=0).T
    return out


# revision 28
# speedup vs baseline: 1.0097x; 1.0097x over previous
"""Distributed multi-head causal attention for 8 TRN2 NeuronCores.

Problem: B=4, T=2048, D=2048, H=16 heads of dk=dv=128.
  out = softmax(mask((q@Wq)(k@Wk)^T / sqrt(dk))) @ (v@Wv) @ Wo

Sharding (2D; all per-core asymmetry lives in host-supplied data so the
SPMD graph is identical on every core):
  core c -> batch b = c//2, head-group g = c%2 (heads 8g..8g+7).
  - QKV projections + attention for (batch b, its 8 heads): fully local.
  - Pair AllGather (replica groups [2b, 2b+1]) exchanges the per-head
    attention outputs (merged^T, bf16) per q-chunk.
  - Output projection: each core computes out^T for its batch for HALF
    the output columns (even core: cols 0..1023, odd: 1024..2047).
  Host reassembles: out[b] = concat(outT_2b, outT_2b+1, axis=0).T

Performance structure (v2):
  - All intermediates (Q^T/K^T per head, V natural) stay RESIDENT IN
    SBUF - no DRAM round trip, no attention-phase input DMAs.
  - Two HWDGE rings: weights/cc/output on nc.sync (SP), activation
    streams on nc.scalar (ACT) - no head-of-line blocking.
  - Attention chunks emitted in order (0, 3, 2, 1); each chunk's output
    projection is emitted 2+ chunk-slots after its attention, so every
    pair-AllGather (~25-30us latency) lands long before its consumer.
    Order: att0 att3 att2 wo3 att1 wo2 wo0 wo1.
  - PE never parks: long warmup covers the initial weight DMA, then the
    V/K/Q projections and attention+wo run as one dense matmul stream
    (keeps the HAM clock gate at K=8/8).

Compute is bf16 on TensorE with f32 PSUM accumulation. Softmax skips the
max-subtraction (scores are ~N(0,1); exp is safe in f32) and obtains the
denominators with an extra ones-matmul so everything stays on TensorE;
causal masking multiplies exp(scores) by a 0/1 triangular tile on the
single diagonal-crossing 128x128 sub-block, and the moving free dim of
diagonal-region matmuls is trimmed to the unmasked columns.
"""
import os
import sys
from contextlib import ExitStack

import numpy as np
import ml_dtypes

import concourse.bass as bass
import concourse.mybir as mybir
import concourse.tile as tile
from concourse import bacc
from concourse.bass_utils import run_bass_kernel_spmd

BF16 = mybir.dt.bfloat16
F32 = mybir.dt.float32

B, T, D = 4, 2048, 2048
H, DK, DV = 16, 128, 128
HG = 8                      # heads per core
N_CORES = 8
QC = 512                    # q-chunk (matmul moving free dim)
NQC = T // QC               # 4
NKB = T // 128              # 16 k-blocks
NDC = D // 128              # 16 contraction chunks
SCALE = 1.0 / np.sqrt(DK)
N_WARM = 290                # dummy matmuls to warm the PE clock gate

_KERNEL_CACHE = {}

# Attention chunk emission order: each wo(x) needs its pair-AllGather
# landed; gathers fire right after att(x) and take ~7-35us, so wo(x) is
# scheduled >=2 chunk-slots later (see the plan at the end of
# build_kernel). att3 goes first: its 16-k-block pipeline is deepest,
# hiding the exp/DVE latency while the phase warms up.
ATT_ORDER = (3, 0, 2, 1)


def build_kernel(causal: bool):
    nc = bacc.Bacc("TRN2", num_devices=N_CORES)

    qT = nc.declare_dram_parameter("qT", [D, T], BF16, isOutput=False)
    kT = nc.declare_dram_parameter("kT", [D, T], BF16, isOutput=False)
    vT = nc.declare_dram_parameter("vT", [D, T], BF16, isOutput=False)
    wq = nc.declare_dram_parameter("wq", [D, HG * DK], BF16, isOutput=False)
    wk = nc.declare_dram_parameter("wk", [D, HG * DK], BF16, isOutput=False)
    wv = nc.declare_dram_parameter("wv", [D, HG * DV], BF16, isOutput=False)
    wo = nc.declare_dram_parameter("wo", [H * DV, D // 2], BF16, isOutput=False)
    tri = nc.declare_dram_parameter("tri", [128, 128], BF16, isOutput=False)
    if not causal:
        maskT = nc.declare_dram_parameter("maskT", [T, T], BF16, isOutput=False)
    outT = nc.declare_dram_parameter("outT", [D // 2, T], F32, isOutput=True)

    # Collective staging (collectives require DRAM in/out). One pair
    # AllGather per q-chunk: in [128, HG, QC], out [2, 128, HG, QC]
    # (slot 0 = even core's heads = global heads 0..7).
    cc_in = {qc: nc.dram_tensor(f"cc_in_{qc}", [128, HG, QC], BF16)
             for qc in range(NQC)}
    cc_out = {qc: nc.dram_tensor(f"cc_out_{qc}", [2, 128, HG, QC], BF16)
              for qc in range(NQC)}
    cc_warm_in = nc.dram_tensor("cc_warm_in", [128, 8], BF16)
    cc_warm_out = nc.dram_tensor("cc_warm_out", [2, 128, 8], BF16)
    pair_groups = [[0, 1], [2, 3], [4, 5], [6, 7]]

    def kb_start(qc, kb):
        """First unmasked q column (within the chunk) for this k-block."""
        if not causal:
            return 0
        return min(max((kb - 4 * qc) * 128, 0), QC)

    with tile.TileContext(nc) as tc, ExitStack() as top:
        ent = top.enter_context
        consts = ent(tc.tile_pool(name="consts", bufs=1))
        # SBUF-resident per-head projections (live for the whole kernel):
        #   q_all/k_all [128(dk), HG, T] = Q^T/K^T per head
        #   v_all [128(krow), HG, NKB, DV] = V natural per head per k-block
        res_pool = ent(tc.tile_pool(name="res", bufs=1))
        # Phase-1-only pools: the weight ring (two zones, wv->wq reuse)
        # and the double-buffered activation streams. Closed before the
        # attention pools open so their SBUF is recycled.
        proj_stack = ExitStack()
        w_pool = proj_stack.enter_context(tc.tile_pool(name="w", bufs=2))
        x_pool = proj_stack.enter_context(tc.tile_pool(name="xs", bufs=2))

        ones_sb = consts.tile([128, 128], BF16)
        nc.vector.memset(ones_sb, 1.0)
        tri_sb = consts.tile([128, 128], BF16)
        nc.sync.dma_start(out=tri_sb, in_=tri[:])

        # Warm the CC stream: the first collective of a NEFF pays the
        # stream barrier + cold-start (~10-30us extra). Fire a tiny
        # dependency-free pair-AllGather now so all of that happens under
        # the projection phase, not under the first real exchange.
        nc.gpsimd.collective_compute(
            "AllGather",
            mybir.AluOpType.bypass,
            ins=[cc_warm_in[:]],
            outs=[cc_warm_out[:]],
            replica_groups=pair_groups,
        )

        q_all = res_pool.tile([128, HG, T], BF16, name="q_all")
        k_all = res_pool.tile([128, HG, T], BF16, name="k_all")
        v_all = res_pool.tile([128, HG, NKB, DV], BF16, name="v_all")

        def weight_tile(pool):
            return pool.tile([128, NDC, HG * 128], BF16, tag="w", name="wtile")

        def weight_slice(w_sb, w_ext, dci):
            nc.sync.dma_start(
                out=w_sb[:, dci, :], in_=w_ext[dci * 128 : (dci + 1) * 128, :]
            )

        # wv first (feeds the first matmuls), wk lands during V proj.
        # wv is the startup-critical load: 4x1MB chunks hit ~78% of DMA
        # peak where 16x256KB slices sat at ~55%, landing ~14us sooner.
        wv_sb = weight_tile(w_pool)
        for c in range(4):
            nc.sync.dma_start(
                out=wv_sb[:, 4 * c : 4 * (c + 1), :],
                in_=wv[4 * c * 128 : 4 * (c + 1) * 128, :].rearrange(
                    "(o p) f -> p o f", p=128
                ),
            )
        wk_sb = weight_tile(w_pool)
        wq_sb = None  # allocated after V proj (reuses wv zone)

        def x_stream(src, qc):
            """[128, NDC, QC] slice of an x^T input, contraction on
            partitions, via the ACT HWDGE ring."""
            xs = x_pool.tile([128, NDC, QC], BF16, tag="xs")
            nc.scalar.dma_start(
                out=xs,
                in_=src[:, qc * QC : (qc + 1) * QC].rearrange(
                    "(o p) f -> p o f", p=128
                ),
            )
            return xs

        # ------------- Phase 1: V, K, Q projections -------------
        # One shared PSUM pool for all three (no pool-boundary bubbles).
        with (
            tc.tile_pool(name="warmps", bufs=1, space="PSUM") as warmps,
            tc.tile_pool(name="ppsum", bufs=4, space="PSUM") as ppsum,
        ):
            # Warm the PE HAM clock gate while the first input DMAs land:
            # dependency-free matmuls on the ones tile into a scratch bank.
            wps = warmps.tile([128, 128], F32)
            for i in range(N_WARM):
                nc.tensor.matmul(
                    wps, lhsT=ones_sb, rhs=ones_sb,
                    start=(i == 0), stop=(i == N_WARM - 1),
                )

            # V natural ([krows, dv], krows on partitions): stationary is
            # the x^T block, the weight columns stream.
            for qv in range(NQC):
                xv = x_stream(vT, qv)
                for dci in (range(4) if qv == 0 else []):
                    weight_slice(wk_sb, wk, dci)
                for kbs in range(4):
                    kb = 4 * qv + kbs
                    if kbs == 3 and qv < NQC - 1:
                        for dci in range(4 * (qv + 1), 4 * (qv + 2)):
                            weight_slice(wk_sb, wk, dci)
                    for nn in range(2):
                        ps = ppsum.tile([128, 512], F32, tag="pp")
                        for dci in range(NDC):
                            nc.tensor.matmul(
                                ps,
                                lhsT=xv[:, dci, kbs * 128 : (kbs + 1) * 128],
                                rhs=wv_sb[:, dci, nn * 512 : (nn + 1) * 512],
                                start=(dci == 0),
                                stop=(dci == NDC - 1),
                            )
                        # drain straight into the resident V tile
                        nc.vector.tensor_copy(
                            out=v_all[:, nn * 4 : (nn + 1) * 4, kb, :], in_=ps
                        )

            # K^T per head ([dk, q]): weight slice stationary, x^T streams.
            wq_sb = weight_tile(w_pool)  # reuses the wv zone
            for qc in range(NQC):
                xs = x_stream(kT, qc)
                for dci in range(4 * qc, 4 * qc + 4):
                    weight_slice(wq_sb, wq, dci)
                for h in range(HG):
                    ps = ppsum.tile([128, QC], F32, tag="pp")
                    for dci in range(NDC):
                        nc.tensor.matmul(
                            ps,
                            lhsT=wk_sb[:, dci, h * 128 : (h + 1) * 128],
                            rhs=xs[:, dci, :],
                            start=(dci == 0),
                            stop=(dci == NDC - 1),
                        )
                    nc.vector.tensor_copy(
                        out=k_all[:, h, qc * QC : (qc + 1) * QC], in_=ps
                    )

            # Q projection, chunks in attention order.
            for qc in ATT_ORDER:
                xs = x_stream(qT, qc)
                for h in range(HG):
                    ps = ppsum.tile([128, QC], F32, tag="pp")
                    for dci in range(NDC):
                        nc.tensor.matmul(
                            ps,
                            lhsT=wq_sb[:, dci, h * 128 : (h + 1) * 128],
                            rhs=xs[:, dci, :],
                            start=(dci == 0),
                            stop=(dci == NDC - 1),
                        )
                    nc.vector.tensor_copy(
                        out=q_all[:, h, qc * QC : (qc + 1) * QC], in_=ps
                    )

        proj_stack.close()

        # ---------- Phase 2+3: attention, pair-AG, output proj ----------
        # wo lands in the SBUF recycled from the weight ring; its 4MB DMA
        # runs under the first attention chunks (first use is emit_wo(3),
        # ~120us into the attention phase).
        wo_pool = ent(tc.tile_pool(name="wop", bufs=1))
        wo_sb = wo_pool.tile([128, NDC, D // 2], BF16, name="wo_sb")
        for dci in range(NDC):
            weight_slice(wo_sb, wo, dci)
        pt_pool = ent(tc.tile_pool(name="pt", bufs=10 if causal else 6))
        racc_pool = ent(tc.tile_pool(name="racc", bufs=4 if causal else 3))
        mstage = ent(tc.tile_pool(name="mstage", bufs=4 if causal else 3))
        rinv_pool = ent(tc.tile_pool(name="rinv", bufs=3 if causal else 2))
        mfq_pool = ent(tc.tile_pool(name="mfq", bufs=2 if causal else 1))
        ob_pool = ent(tc.tile_pool(name="ob", bufs=4))
        gm_pool = ent(tc.tile_pool(name="gm", bufs=2)) if not causal else None
        spsum = ent(tc.tile_pool(name="spsum", bufs=2, space="PSUM"))
        opsum = ent(tc.tile_pool(name="opsum", bufs=2, space="PSUM"))
        rpsum = ent(tc.tile_pool(name="rpsum", bufs=2, space="PSUM"))
        wpsum = ent(tc.tile_pool(name="wpsum", bufs=2, space="PSUM"))

        def load_gm(qc):
            if causal:
                return None
            gm = gm_pool.tile([128, NKB, QC], BF16, tag="gm")
            nc.scalar.dma_start(
                out=gm,
                in_=maskT[:, qc * QC : (qc + 1) * QC].rearrange(
                    "(o p) f -> p o f", p=128
                ),
            )
            return gm

        def att_head(qc, h, gm):
            nkb = 4 * (qc + 1) if causal else NKB
            ngrp = (nkb + 3) // 4
            # Process the diagonal-crossing k-group FIRST: its thin
            # (128..512-wide) s->exp->o chains then overlap the dense
            # full-width blocks instead of bunching at the head's end.
            # PSUM accumulation is order-independent; the group-first
            # block always has j0=0 either way.
            grp_order = ([ngrp - 1] + list(range(ngrp - 1))) if causal else \
                list(range(ngrp))
            kb_order = [4 * g + j for g in grp_order for j in range(4)
                        if 4 * g + j < nkb]
            if True:
                o_ps = opsum.tile([128, QC], F32, tag="opsum")
                r_ps = rpsum.tile([128, QC], F32, tag="rpsum")
                racc = None
                for kbi, kb in enumerate(kb_order):
                    j0 = kb_start(qc, kb)  # first live q col in chunk
                    s_ps = spsum.tile([128, QC], F32, tag="spsum")
                    nc.tensor.matmul(
                        s_ps[:, j0:],
                        lhsT=k_all[:, h, kb * 128 : (kb + 1) * 128],
                        rhs=q_all[:, h, qc * QC + j0 : (qc + 1) * QC],
                        start=True,
                        stop=True,
                    )
                    pt = pt_pool.tile([128, QC], BF16, tag="pt")
                    nc.scalar.activation(
                        out=pt[:, j0:],
                        in_=s_ps[:, j0:],
                        func=mybir.ActivationFunctionType.Exp,
                        scale=float(SCALE),
                    )
                    if causal:
                        if j0 < QC and kb - 4 * qc >= 0:
                            # mask the diagonal-crossing 128 columns
                            nc.vector.tensor_mul(
                                out=pt[:, j0 : j0 + 128],
                                in0=pt[:, j0 : j0 + 128],
                                in1=tri_sb,
                            )
                    else:
                        nc.vector.tensor_mul(out=pt, in0=pt, in1=gm[:, kb, :])
                    nc.tensor.matmul(
                        o_ps[:, j0:],
                        lhsT=v_all[:, h, kb, :],
                        rhs=pt[:, j0:],
                        start=(kbi == 0),
                        stop=(kbi == nkb - 1),
                    )
                    # Denominators: sum groups of 4 exp-blocks on DVE
                    # (bf16), then ONE full-width ones-matmul per group -
                    # a third of the attention PE time used to go to a
                    # per-block ones-matmul. The group's first block always
                    # has j0=0, so racc is fully initialized.
                    if kbi % 4 == 0:
                        racc = racc_pool.tile([128, QC], BF16, tag="racc")
                        nc.vector.tensor_copy(out=racc, in_=pt)
                    else:
                        nc.vector.tensor_add(
                            out=racc[:, j0:], in0=racc[:, j0:], in1=pt[:, j0:]
                        )
                    if kbi % 4 == 3 or kbi == nkb - 1:
                        nc.tensor.matmul(
                            r_ps,
                            lhsT=ones_sb,
                            rhs=racc,
                            start=(kbi // 4 == 0),
                            stop=(kbi // 4 == ngrp - 1),
                        )
                # 1/r on DVE. The exact reciprocal (~6 cycles/elem, 3.4us
                # per head) held the rpsum bank hostage and stalled the PE;
                # the single-instruction NR-seeded approximation (~51 ULP)
                # is ~5x faster. (Scalar-engine Ln+Exp(-x) is no better:
                # exp and ln land in different ACT tables and each head
                # pays two 1.3us ACT_TABLE_LOADs.)
                rinv = rinv_pool.tile([128, QC], F32, tag="rinv")
                nc.vector.reciprocal_approx_fast(out=rinv, in_=r_ps)
                msb = mstage.tile([128, QC], BF16, tag="mstage")
                nc.vector.tensor_mul(out=msb, in0=o_ps, in1=rinv)
                nc.sync.dma_start(out=cc_in[qc][:, h, :], in_=msb)
                if h == HG - 1:
                    nc.gpsimd.collective_compute(
                        "AllGather",
                        mybir.AluOpType.bypass,
                        ins=[cc_in[qc][:]],
                        outs=[cc_out[qc][:]],
                        replica_groups=pair_groups,
                    )

        def wo_load(qc):
            # Prefetch the gathered heads for wo(qc), a full slot early.
            mfq = mfq_pool.tile([128, H, QC], BF16, tag="mfq")
            nc.sync.dma_start(out=mfq[:, :HG, :], in_=cc_out[qc][0])
            nc.sync.dma_start(out=mfq[:, HG:, :], in_=cc_out[qc][1])
            return mfq

        def wo_col(qc, col, mfq):
            w_ps = wpsum.tile([128, QC], F32, tag="wpsum")
            for hv in range(H):
                nc.tensor.matmul(
                    w_ps,
                    lhsT=wo_sb[:, hv, col * 128 : (col + 1) * 128],
                    rhs=mfq[:, hv, :],
                    start=(hv == 0),
                    stop=(hv == H - 1),
                )
            # drain on the Scalar engine - DVE is loaded with the softmax
            # element-wise work, ACT is idle during wo stretches
            ob = ob_pool.tile([128, QC], F32, tag="ob")
            nc.scalar.activation(
                out=ob, in_=w_ps, func=mybir.ActivationFunctionType.Copy
            )
            # outT goes out on the ACT HWDGE ring: the SP ring carries
            # cc_in staging + mfq loads and outT bursts were head-of-line
            # blocking them.
            nc.scalar.dma_start(
                out=outT[
                    col * 128 : (col + 1) * 128,
                    qc * QC : (qc + 1) * QC,
                ],
                in_=ob,
            )

        # Plan: att3 att0 att2 run back-to-back (their gathers fire and
        # land under later chunks); att1's heads interleave with wo3's
        # columns so the exp pipeline never drains ahead of a pure-matmul
        # stretch; remaining wo chunks run ordered by gather-land time.
        NCOL = D // 2 // 128
        gm3 = load_gm(3)
        for h in range(HG):
            att_head(3, h, gm3)
        gm0 = load_gm(0)
        for h in range(HG):
            att_head(0, h, gm0)
        mfq3 = wo_load(3)
        gm2 = load_gm(2)
        for h in range(HG):
            att_head(2, h, gm2)
        mfq0 = wo_load(0)
        gm1 = load_gm(1)
        for h in range(HG):
            att_head(1, h, gm1)
            wo_col(3, h, mfq3)
        mfq2 = wo_load(2)
        for col in range(NCOL):
            wo_col(0, col, mfq0)
        mfq1 = wo_load(1)
        for col in range(NCOL):
            wo_col(2, col, mfq2)
        for col in range(NCOL):
            wo_col(1, col, mfq1)

    nc.compile()
    return nc


def kernel(q, k, v, mask, Wq, Wk, Wv, Wo):
    q = np.asarray(q)
    k = np.asarray(k)
    v = np.asarray(v)
    mask = np.asarray(mask)
    causal = bool(np.array_equal(mask, np.tril(np.ones((T, T), dtype=bool))))

    if causal not in _KERNEL_CACHE:
        _KERNEL_CACHE[causal] = build_kernel(causal)
    nc = _KERNEL_CACHE[causal]

    bf = ml_dtypes.bfloat16
    Wq_b = np.asarray(Wq).astype(bf)
    Wk_b = np.asarray(Wk).astype(bf)
    Wv_b = np.asarray(Wv).astype(bf)
    Wo_b = np.asarray(Wo).astype(bf)
    i = np.arange(128)
    tri_np = (i[None, :] >= i[:, None]).astype(bf)  # tri[k, j] = j >= k
    maskT_np = None if causal else np.ascontiguousarray(mask.T).astype(bf)

    in_maps = []
    for c in range(N_CORES):
        b, g = c // 2, c % 2
        m = {
            "qT": np.ascontiguousarray(q[b].T).astype(bf),
            "kT": np.ascontiguousarray(k[b].T).astype(bf),
            "vT": np.ascontiguousarray(v[b].T).astype(bf),
            "wq": np.ascontiguousarray(Wq_b[:, g * 1024 : (g + 1) * 1024]),
            "wk": np.ascontiguousarray(Wk_b[:, g * 1024 : (g + 1) * 1024]),
            "wv": np.ascontiguousarray(Wv_b[:, g * 1024 : (g + 1) * 1024]),
            "wo": np.ascontiguousarray(Wo_b[:, g * 1024 : (g + 1) * 1024]),
            "tri": tri_np,
        }
        if not causal:
            m["maskT"] = maskT_np
        in_maps.append(m)

    trace = bool(os.environ.get("BASS_KERNEL_TRACE")) and (
        "antenv.axon_hooks" in sys.modules
    )
    res = run_bass_kernel_spmd(nc, in_maps, list(range(N_CORES)), trace=trace)
    if trace and res.exec_time_ns is not None:
        print(f"HW exec time: {res.exec_time_ns} ns")
        kernel.last_exec_time_ns = res.exec_time_ns
        kernel.last_results = res

    out = np.empty((B, T, D), dtype=np.float32)
    for b in range(B):
        top = res.results[2 * b]["outT"]        # cols 0..1023, [1024, 2048]
        bot = res.results[2 * b + 1]["outT"]    # cols 1024..2047
        out[b] = np.concatenate([top, bot], axis=0).T
    return out
